# revision 19
# baseline (speedup 1.0000x reference)
"""AFT-Full attention kernel for 8 TRN2 NeuronCores.

Data-parallel over batch B=8 (one batch element per core). Per core:
  Q = x_q @ wq + wq_b          [2048, 256]
  K = x_kv @ wk + wk_b         [2048, 256]
  V = x_kv @ wv + wv_b         [2048, 256]
  num = exp(bias) @ (exp(K)*V) [2048, 256]
  den = exp(bias) @ exp(K)     [2048, 256]
  Yt  = sigmoid(Q) * num / den
  out = Yt @ f2_w + f2_b       [2048, 256]

The kernel is HBM-read-bound on the load side (~35 MB of f32 inputs at
the ~358 GB/s per-NC limit is a ~100 us floor) and PE-row-bound on the
compute side. Phases run SEQUENTIALLY (A: all K/V groups, B: all Q
chunks, C: all num/den chunks) -- interleaving B/C per chunk was tried
and measured ~7 us/chunk SLOWER from cross-phase psum/ACT contention.
The DMA queue order matches consumption, and no dma_start ever waits on
a staging-slot semaphore (a waiting DMA blocks the whole SWDGE queue):
bias staging uses 16 quarter-chunk tiles with 12 slots so the reuse
waits land after their targets are long dead.

Precision split (hard-won): num is a SIGNED accumulation, so independent
per-element quantization errors pass through at full strength (no sqrt-N
averaging) -- every operand on the num path (x_kv, wk/wv, exp(K)*V,
exp(bias)) must stay bf16. den is an all-positive accumulation and Q only
feeds a sigmoid gate; both tolerate fp8e4:
  - den runs fp8 MatmulPerfMode.DoubleRow (2 k-tiles/instruction, halves
    den's PE rows): exp(bias) gets an extra fp8 copy (DVE) and exp(K) an
    extra fp8 ACT write.
  - Q runs fp8 DoubleRow: x_q^T tiles are written fp8 by the existing
    PSUM->SBUF transpose copies (free); wq is loaded bf16 and converted
    once on ACT (fp8 DMA casts run at HALF the bf16 DMA rate, so no bulk
    fp8 loads).
  - Everything else (K/V, num, f2, all transposes) is bf16. The output
    is stored bf16 (saves HBM writes; host casts back to f32).

Other notes:
- Transposes are regular matmuls vs an identity (stationary = data), NOT
  is_transpose: transpose-mode does not count as PE-busy for the HAM
  clock gate and previously left the PE throttled at 1.2 GHz.
- The epilogue 1+exp(-Q) runs on ACT; the rest (mul/recip/mul) on DVE.
- Chunks 0-2 defer their f2 matmuls into the next chunk's phase C (PE is
  strict FIFO; it would otherwise idle behind the DVE epilogue at the
  chunk boundary). Chunk 3 pipelines epilogue/f2/store in s-halves to
  shorten the end-of-kernel tail.
"""

import os
import numpy as np
from contextlib import ExitStack

import concourse.bass as bass
import concourse.tile as tile
from concourse import bacc, mybir
from concourse.bass_utils import run_bass_kernel_spmd
from concourse.masks import make_identity

F32 = mybir.dt.float32
BF16 = mybir.dt.bfloat16
FP8 = mybir.dt.float8e4

S = 2048   # n_q
T = 2048   # n_kv
D = 1024   # d_q == d_kv
H = 256    # hidden
G = 256    # output dim
P = 128    # partitions
SCH = 512  # s-chunk for phases B/C (one PSUM bank of fp32)
NSB = SCH // P       # 4 row-blocks per chunk
NCH = S // SCH       # 4 chunks
NT = T // P          # 16 t row-blocks
NG = NT // 2         # 8 groups of 2 t-blocks
ND = D // P          # 8 d tiles
NDD = ND // 2        # 4 d-tile pairs (DoubleRow)
NHB = H // P         # 2 h blocks
TQ = T // 4          # bias quarter length along t

DR = mybir.MatmulPerfMode.DoubleRow
AFT = mybir.ActivationFunctionType


def _build(use_wq_b, use_wk_b, use_wv_b, use_f2_b):
    """Build the per-core Bass graph. Returns the compiled Bacc."""
    nc = bacc.Bacc(
        "TRN2",
        target_bir_lowering=False,
        debug=False,
        enable_asserts=False,
        num_devices=8,
    )

    x_q = nc.declare_dram_parameter("x_q", [S, D], F32, isOutput=False)
    x_kv = nc.declare_dram_parameter("x_kv", [T, D], F32, isOutput=False)
    bias = nc.declare_dram_parameter("bias", [S, T], F32, isOutput=False)
    wq_w = nc.declare_dram_parameter("wq_w", [D, H], F32, isOutput=False)
    wk_w = nc.declare_dram_parameter("wk_w", [D, H], F32, isOutput=False)
    wv_w = nc.declare_dram_parameter("wv_w", [D, H], F32, isOutput=False)
    f2_w = nc.declare_dram_parameter("f2_w", [H, G], F32, isOutput=False)
    wq_b = nc.declare_dram_parameter("wq_b", [1, H], F32, isOutput=False) if use_wq_b else None
    wk_b = nc.declare_dram_parameter("wk_b", [1, H], F32, isOutput=False) if use_wk_b else None
    wv_b = nc.declare_dram_parameter("wv_b", [1, H], F32, isOutput=False) if use_wv_b else None
    f2_b = nc.declare_dram_parameter("f2_b", [1, G], F32, isOutput=False) if use_f2_b else None
    out = nc.declare_dram_parameter("out", [S, G], BF16, isOutput=True)

    # DRAM views with the chunk/block structure exposed.
    xq_v = x_q[:].rearrange("(c sb p) d -> c p sb d", p=P, sb=NSB)     # [NCH,P,NSB,D]
    xkv_v = x_kv[:].rearrange("(g j p) d -> g p j d", p=P, j=2)        # [NG,P,2,D]
    bias_v = bias[:].rearrange("(c sb p) (q t) -> c q p sb t", p=P, sb=NSB, q=4)
    out_v = out[:].rearrange("(c sb p) g -> c p sb g", p=P, sb=NSB)

    with tile.TileContext(nc) as tc, ExitStack() as ctx:
        consts = ctx.enter_context(tc.tile_pool(name="consts", bufs=1))

        ident = consts.tile([P, P], BF16)

        # Staging pools sized so the SWDGE queue never waits on a slot.
        xkv_nat = ctx.enter_context(tc.tile_pool(name="xkv_nat", bufs=2 * NG))
        xq_nat = ctx.enter_context(tc.tile_pool(name="xq_nat", bufs=NCH))
        bias_nat = ctx.enter_context(tc.tile_pool(name="bias_nat", bufs=12))
        xT_sb = ctx.enter_context(tc.tile_pool(name="xT_sb", bufs=8))
        xqT2_sb = ctx.enter_context(tc.tile_pool(name="xqT2_sb", bufs=8))
        expw_sb = ctx.enter_context(tc.tile_pool(name="expw_sb", bufs=6))
        expw8_sb = ctx.enter_context(tc.tile_pool(name="expw8_sb", bufs=3))
        epi = ctx.enter_context(tc.tile_pool(name="epi", bufs=3))
        out_sb_pool = ctx.enter_context(tc.tile_pool(name="out_sb", bufs=2))

        # ---- DMA emission order == consumption order ----
        def load_xkv_j(g, j):
            # Per-(group, j) half tiles: halves the first-transpose latency
            # at kernel start.
            t_ = xkv_nat.tile([P, D], BF16, tag="xkv_nat", name="xkv_nat")
            nc.gpsimd.dma_start(t_[:], xkv_v[g][:, j, :])
            return t_

        def load_xq(c):
            t_ = xq_nat.tile([P, NSB, D], BF16, tag="xq_nat", name="xq_nat")
            nc.gpsimd.dma_start(t_[:], xq_v[c])
            return t_

        xkv_tiles = {}
        for j_ in range(2):
            xkv_tiles[(0, j_)] = load_xkv_j(0, j_)
        # Identity built after the first loads so it does not delay the
        # first DMA on the gpsimd queue.
        make_identity(nc, ident[:])
        # wk|wv fused along the free dim -> K/V matmuls share one N=512 rhs.
        # Loaded right after g0 so the first a_mm starts early.
        wkv_sb = consts.tile([P, ND, 2 * H], BF16, tag="w_wkv", name="wkv_sb")
        nc.gpsimd.dma_start(wkv_sb[:, :, 0:H], wk_w[:].rearrange("(dt p) h -> p dt h", p=P))
        nc.gpsimd.dma_start(wkv_sb[:, :, H : 2 * H], wv_w[:].rearrange("(dt p) h -> p dt h", p=P))
        for g_ in range(1, NG):
            for j_ in range(2):
                xkv_tiles[(g_, j_)] = load_xkv_j(g_, j_)
        wq_bf = consts.tile([P, ND, H], BF16, tag="w_wq", name="wq_bf")
        nc.gpsimd.dma_start(wq_bf[:], wq_w[:].rearrange("(dt p) h -> p dt h", p=P))
        wq_sb = consts.tile([P, ND, H], FP8, tag="w_wq8", name="wq_sb")
        f2_sb = consts.tile([P, NHB, G], BF16, tag="w_f2")
        nc.gpsimd.dma_start(f2_sb[:], f2_w[:].rearrange("(ht p) g -> p ht g", p=P))
        # Load order: xq c0-c2, bias c0, xq c3, bias c1-c3. Phase C(0) can
        # then start right as B(0-2) finish; B(3) is emitted between C(0)
        # and C(1) and fills C(0)'s epilogue latency.
        bias_tiles = {}

        def load_bias(c):
            for q_ in range(4):
                t_ = bias_nat.tile([P, NSB, TQ], BF16, tag="bias_nat", name="bias_nat")
                nc.gpsimd.dma_start(t_[:], bias_v[c][q_])
                bias_tiles[(c, q_)] = t_

        xq_tiles = {c_: load_xq(c_) for c_ in range(NCH - 1)}
        load_bias(0)
        xq_tiles[NCH - 1] = load_xq(NCH - 1)
        for c_ in range(1, NCH):
            load_bias(c_)

        bias_vecs = {}
        ones_row = None
        if any(b is not None for b in (wq_b, wk_b, wv_b, f2_b)):
            ones_row = consts.tile([1, SCH], BF16)
            nc.gpsimd.memset(ones_row[:], 1.0)
            for name, b in (("wq", wq_b), ("f2", f2_b)):
                if b is not None:
                    bt = consts.tile([1, H], BF16, tag=f"b_{name}", name="bt")
                    nc.gpsimd.dma_start(bt[:], b[:])
                    bias_vecs[name] = bt
            if wk_b is not None or wv_b is not None:
                bkv = consts.tile([1, 2 * H], BF16, tag="b_kv", name="bkv")
                nc.gpsimd.memset(bkv[:], 0.0)
                if wk_b is not None:
                    nc.gpsimd.dma_start(bkv[:, 0:H], wk_b[:])
                if wv_b is not None:
                    nc.gpsimd.dma_start(bkv[:, H : 2 * H], wv_b[:])
                bias_vecs["kv"] = bkv

        # Long-lived activations.
        enq_pool = ctx.enter_context(tc.tile_pool(name="enq", bufs=NHB))
        enq = [enq_pool.tile([P, S], BF16, tag="enq", name="enq") for _ in range(NHB)]
        ek_pool = ctx.enter_context(tc.tile_pool(name="expk", bufs=NG))
        ek8_pool = ctx.enter_context(tc.tile_pool(name="expk8", bufs=NG))
        ekv_pool = ctx.enter_context(tc.tile_pool(name="ekv", bufs=NG))
        # expk/ekv: [P, 2H] bf16, j-major (lhsT slices for the bf16 num mm).
        # expk8: [P, 2, H] fp8, k-tile-pair layout for the den DoubleRow mm.
        expk = [ek_pool.tile([P, 2 * H], BF16, tag="expk", name="expk") for _ in range(NG)]
        expk8 = [ek8_pool.tile([P, 2, H], FP8, tag="expk8", name="expk8") for _ in range(NG)]
        ekv = [ekv_pool.tile([P, 2 * H], BF16, tag="ekv", name="ekv") for _ in range(NG)]

        # ---------------- Phases A, B, C ----------------
        # One shared PSUM layout (tr 2 + acc 2 + num/den 4 = 8 banks); f2
        # borrows from acc, whose ps_q/ps_kv tiles are drained by then.
        with (
            tc.tile_pool(name="psum_tr", bufs=2, space="PSUM") as psum_tr,
            tc.tile_pool(name="psum_acc", bufs=2, space="PSUM") as psum_acc,
            tc.tile_pool(name="psum_nd", bufs=4, space="PSUM") as psum_nd,
        ):
            # Phase A: K/V, exp(K), exp(K)*V, two t-blocks per group.
            def phase_a(g):
                ps_kv = [psum_acc.tile([P, 2 * H], F32, tag="acc", name="ps_kv") for _ in range(2)]
                xkvT = {}

                def a_tr(d, xkvT=xkvT):
                    ps = psum_tr.tile([P, SCH], F32, tag="tr", name="ps_tr")
                    for j in range(2):
                        nc.tensor.matmul(
                            ps[:, j * P : (j + 1) * P],
                            xkv_tiles[(g, j)][:, d * P : (d + 1) * P],
                            ident[:],
                        )
                    t_ = xT_sb.tile([P, 2 * P], BF16, tag="xkvT", name="xkvT")
                    nc.vector.tensor_copy(t_[:], ps[:, 0 : 2 * P])
                    xkvT[d] = t_

                def a_mm(d, ps_kv=ps_kv, xkvT=xkvT):
                    for j in range(2):
                        nc.tensor.matmul(
                            ps_kv[j][:],
                            xkvT[d][:, j * P : (j + 1) * P],
                            wkv_sb[:, d, :],
                            start=(d == 0),
                            stop=(d == ND - 1 and "kv" not in bias_vecs),
                        )

                for d in range(ND):
                    a_tr(d)
                    if d >= 3:
                        a_mm(d - 3)
                a_mm(ND - 3)
                a_mm(ND - 2)
                a_mm(ND - 1)
                if "kv" in bias_vecs:
                    for j in range(2):
                        nc.tensor.matmul(
                            ps_kv[j][:],
                            ones_row[:, 0:P],
                            bias_vecs["kv"][:],
                            start=False,
                            stop=True,
                        )
                for j in range(2):
                    nc.scalar.activation(
                        expk[g][:, j * H : (j + 1) * H], ps_kv[j][:, 0:H], AFT.Exp
                    )
                    nc.scalar.activation(expk8[g][:, j, :], ps_kv[j][:, 0:H], AFT.Exp)
                    nc.vector.tensor_mul(
                        ekv[g][:, j * H : (j + 1) * H],
                        expk[g][:, j * H : (j + 1) * H],
                        ps_kv[j][:, H : 2 * H],
                    )

            # Phase B: Q^T (fp8 DoubleRow) + exp(-Q), one chunk at a time.
            def phase_b(c):
                nat = xq_tiles[c]
                ps_q = [psum_acc.tile([P, SCH], F32, tag="acc", name="ps_q") for _ in range(NHB)]
                xqT2 = {}

                def b_tr(d, nat=nat, xqT2=xqT2):
                    ps = psum_tr.tile([P, SCH], F32, tag="tr", name="ps_trq")
                    for sb in range(NSB):
                        nc.tensor.matmul(
                            ps[:, sb * P : (sb + 1) * P],
                            nat[:, sb, d * P : (d + 1) * P],
                            ident[:],
                        )
                    if d % 2 == 0:
                        xqT2[d // 2] = xqT2_sb.tile(
                            [P, 2, SCH], FP8, tag="xqT2", name="xqT2"
                        )
                    nc.vector.tensor_copy(xqT2[d // 2][:, d % 2, :], ps[:])

                def b_mm(dd, ps_q=ps_q, xqT2=xqT2):
                    for hb in range(NHB):
                        nc.tensor.matmul(
                            ps_q[hb][:],
                            wq_sb[:, 2 * dd : 2 * dd + 2, hb * P : (hb + 1) * P],
                            xqT2[dd][:],
                            start=(dd == 0),
                            stop=(dd == NDD - 1 and "wq" not in bias_vecs),
                            perf_mode=DR,
                        )

                for d in range(ND):
                    b_tr(d)
                    if d == 3:
                        b_mm(0)
                    elif d == 5:
                        b_mm(1)
                    elif d == 7:
                        b_mm(2)
                b_mm(NDD - 1)
                for hb in range(NHB):
                    if "wq" in bias_vecs:
                        nc.tensor.matmul(
                            ps_q[hb][:],
                            bias_vecs["wq"][:, hb * P : (hb + 1) * P],
                            ones_row[:],
                            start=False,
                            stop=True,
                        )
                    nc.scalar.activation(
                        enq[hb][:, c * SCH : (c + 1) * SCH],
                        ps_q[hb][:],
                        AFT.Exp,
                        scale=-1.0,
                    )

            def emit_f2(ytT, c):
                # Deferred f2 projection + store for chunks 0-2. PSUM borrowed
                # from the acc pool (its ps_q/ps_kv tiles are drained by now).
                out_sb = out_sb_pool.tile([P, NSB, G], BF16, tag="out_sb", name="out_sb")
                for sb in range(NSB):
                    ps_f = psum_acc.tile([P, SCH], F32, tag="acc", name="ps_f")
                    for hb in range(NHB):
                        nc.tensor.matmul(
                            ps_f[:, 0:G],
                            ytT[hb][:, sb * P : (sb + 1) * P],
                            f2_sb[:, hb, :],
                            start=(hb == 0),
                            stop=(hb == NHB - 1 and "f2" not in bias_vecs),
                        )
                    if "f2" in bias_vecs:
                        nc.tensor.matmul(
                            ps_f[:, 0:G],
                            ones_row[:, 0:P],
                            bias_vecs["f2"][:],
                            start=False,
                            stop=True,
                        )
                    nc.vector.tensor_copy(out_sb[:, sb, :], ps_f[:, 0:G])
                nc.sync.dma_start(out_v[c], out_sb[:])

            pending_f2 = [None]

            # Phase C: exp(bias)^T, num (bf16) / den (fp8 DR), epilogue.
            def phase_c(c):
                ps_num = [psum_nd.tile([P, SCH], F32, tag="nd", name="ps_num") for _ in range(NHB)]
                ps_den = [psum_nd.tile([P, SCH], F32, tag="nd", name="ps_den") for _ in range(NHB)]
                expw = []
                expw8 = {}

                def c_tr(tb, expw=expw, expw8=expw8):
                    nath = bias_tiles[(c, tb // 4)]
                    tloc = (tb % 4) * P
                    ps = psum_tr.tile([P, SCH], F32, tag="tr", name="ps_trc")
                    for sb in range(NSB):
                        nc.tensor.matmul(
                            ps[:, sb * P : (sb + 1) * P],
                            nath[:, sb, tloc : tloc + P],
                            ident[:],
                        )
                    sbuf = expw_sb.tile([P, SCH], BF16, tag="expw", name="expw")
                    nc.scalar.activation(sbuf[:], ps[:], AFT.Exp)
                    expw.append(sbuf)
                    # fp8 copy for the den DoubleRow matmul.
                    g = tb // 2
                    if tb % 2 == 0:
                        expw8[g] = expw8_sb.tile([P, 2, SCH], FP8, tag="expw8", name="expw8")
                    nc.vector.tensor_copy(expw8[g][:, tb % 2, :], sbuf[:])

                def c_num(tb, ps_num=ps_num, expw=expw):
                    g, j = tb // 2, tb % 2
                    rhs = expw[tb][:]
                    for hb in range(NHB):
                        lo = j * H + hb * P
                        nc.tensor.matmul(
                            ps_num[hb][:],
                            ekv[g][:, lo : lo + P],
                            rhs,
                            start=(tb == 0),
                            stop=(tb == NT - 1),
                        )

                def c_den(g, ps_den=ps_den, expw8=expw8):
                    for hb in range(NHB):
                        nc.tensor.matmul(
                            ps_den[hb][:],
                            expk8[g][:, :, hb * P : (hb + 1) * P],
                            expw8[g][:],
                            start=(g == 0),
                            stop=(g == NG - 1),
                            perf_mode=DR,
                        )

                for tb in range(NT):
                    c_tr(tb)
                    if tb == 3 and pending_f2[0] is not None:
                        emit_f2(*pending_f2[0])
                        pending_f2[0] = None
                    if tb >= 2:
                        c_num(tb - 2)
                    if tb >= 3 and tb % 2 == 1:
                        c_den((tb - 3) // 2)
                c_num(NT - 2)
                c_num(NT - 1)
                c_den(NG - 1)

                if c < NCH - 1:
                    # Epilogue: Yt^T = num^T / (den^T * (1 + exp(-Q)^T)).
                    ytT = []
                    for hb in range(NHB):
                        t1 = epi.tile([P, SCH], BF16, tag="t1", name="t1")
                        nc.scalar.add(t1[:], enq[hb][:, c * SCH : (c + 1) * SCH], 1.0)
                        d2 = epi.tile([P, SCH], F32, tag="d2", name="d2", bufs=2)
                        nc.vector.tensor_mul(d2[:], t1[:], ps_den[hb][:])
                        rec = epi.tile([P, SCH], F32, tag="rec", name="rec", bufs=2)
                        nc.vector.reciprocal_approx_fast(rec[:], d2[:])
                        yt = epi.tile([P, SCH], BF16, tag="yt", name="yt", bufs=4)
                        nc.vector.tensor_mul(yt[:], rec[:], ps_num[hb][:])
                        ytT.append(yt)
                    pending_f2[0] = (ytT, c)
                else:
                    # Last chunk: pipeline epilogue/f2/store in s-halves to
                    # shorten the end-of-kernel tail.
                    for h in range(2):
                        sl = slice(h * 2 * P, (h + 1) * 2 * P)
                        yth = []
                        for hb in range(NHB):
                            t1 = epi.tile([P, 2 * P], BF16, tag="t1h", name="t1h")
                            nc.scalar.add(
                                t1[:],
                                enq[hb][:, c * SCH + h * 2 * P : c * SCH + (h + 1) * 2 * P],
                                1.0,
                            )
                            d2 = epi.tile([P, 2 * P], F32, tag="d2h", name="d2h", bufs=2)
                            nc.vector.tensor_mul(d2[:], t1[:], ps_den[hb][:, sl])
                            rec = epi.tile([P, 2 * P], F32, tag="rech", name="rech", bufs=2)
                            nc.vector.reciprocal_approx_fast(rec[:], d2[:])
                            yt = epi.tile([P, 2 * P], BF16, tag="yth", name="yth")
                            nc.vector.tensor_mul(yt[:], rec[:], ps_num[hb][:, sl])
                            yth.append(yt)
                        out_sb = out_sb_pool.tile([P, 2, G], BF16, tag="out_h", name="out_h")
                        for k in range(2):
                            ps_f = psum_acc.tile([P, SCH], F32, tag="acc", name="ps_fh")
                            for hb in range(NHB):
                                nc.tensor.matmul(
                                    ps_f[:, 0:G],
                                    yth[hb][:, k * P : (k + 1) * P],
                                    f2_sb[:, hb, :],
                                    start=(hb == 0),
                                    stop=(hb == NHB - 1 and "f2" not in bias_vecs),
                                )
                            if "f2" in bias_vecs:
                                nc.tensor.matmul(
                                    ps_f[:, 0:G],
                                    ones_row[:, 0:P],
                                    bias_vecs["f2"][:],
                                    start=False,
                                    stop=True,
                                )
                            nc.vector.tensor_copy(out_sb[:, k, :], ps_f[:, 0:G])
                        nc.sync.dma_start(out_v[c][:, 2 * h : 2 * h + 2, :], out_sb[:])

            for g_ in range(NG):
                phase_a(g_)
            # wq bf16 -> fp8 conversion: emitted after phase A so it does not
            # block the ACT FIFO (its input DMA lands ~40us in).
            nc.scalar.copy(wq_sb[:], wq_bf[:])
            # B(0-2), C(0), then B(3) -- B(3) fills C(0)'s epilogue latency
            # and its xq arrives after bias c0 in the load stream.
            for c_ in range(NCH - 1):
                phase_b(c_)
            phase_c(0)
            phase_b(NCH - 1)
            for c_ in range(1, NCH):
                phase_c(c_)

    nc.compile()
    return nc


_CACHE = {}


def _get_nc(use_wq_b, use_wk_b, use_wv_b, use_f2_b):
    key = (use_wq_b, use_wk_b, use_wv_b, use_f2_b)
    if key not in _CACHE:
        _CACHE[key] = _build(*key)
    return _CACHE[key]


def kernel(x_q, x_kv, bias, wq_w, wq_b, wk_w, wk_b, wv_w, wv_b, f2_w, f2_b,
           _trace=False, _trace_kwargs=None):
    x_q = np.ascontiguousarray(np.asarray(x_q, dtype=np.float32))
    x_kv = np.ascontiguousarray(np.asarray(x_kv, dtype=np.float32))
    bias = np.ascontiguousarray(np.asarray(bias, dtype=np.float32))
    wq_w = np.ascontiguousarray(np.asarray(wq_w, dtype=np.float32))
    wk_w = np.ascontiguousarray(np.asarray(wk_w, dtype=np.float32))
    wv_w = np.ascontiguousarray(np.asarray(wv_w, dtype=np.float32))
    f2_w = np.ascontiguousarray(np.asarray(f2_w, dtype=np.float32))
    wq_b = np.asarray(wq_b, dtype=np.float32)
    wk_b = np.asarray(wk_b, dtype=np.float32)
    wv_b = np.asarray(wv_b, dtype=np.float32)
    f2_b = np.asarray(f2_b, dtype=np.float32)

    use_b = tuple(bool(np.any(b)) for b in (wq_b, wk_b, wv_b, f2_b))
    nc = _get_nc(*use_b)

    n_cores = 8
    in_maps = []
    for i in range(n_cores):
        m = {
            "x_q": x_q[i],
            "x_kv": x_kv[i],
            "bias": bias[i],
            "wq_w": wq_w,
            "wk_w": wk_w,
            "wv_w": wv_w,
            "f2_w": f2_w,
        }
        if use_b[0]:
            m["wq_b"] = wq_b.reshape(1, H)
        if use_b[1]:
            m["wk_b"] = wk_b.reshape(1, H)
        if use_b[2]:
            m["wv_b"] = wv_b.reshape(1, H)
        if use_b[3]:
            m["f2_b"] = f2_b.reshape(1, G)
        in_maps.append(m)

    if not _trace:
        # The NTFF trace hook is unavailable outside the dev harness; make
        # sure a stray BASS_TRACE env var cannot route us onto that path.
        os.environ["BASS_NEVER_TRACE"] = "1"
    else:
        os.environ.pop("BASS_NEVER_TRACE", None)
    res = run_bass_kernel_spmd(
        nc, in_maps, list(range(n_cores)), trace=_trace, **(_trace_kwargs or {})
    )
    out = np.stack(
        [np.asarray(res.results[i]["out"]).astype(np.float32) for i in range(n_cores)],
        axis=0,
    )
    if _trace:
        return out, res
    return out


# revision 21
# speedup vs baseline: 1.0854x; 1.0854x over previous
"""AFT-Full attention kernel for 8 TRN2 NeuronCores.

Data-parallel over batch B=8 (one batch element per core). Per core:
  Q = x_q @ wq + wq_b          [2048, 256]
  K = x_kv @ wk + wk_b         [2048, 256]
  V = x_kv @ wv + wv_b         [2048, 256]
  num = exp(bias) @ (exp(K)*V) [2048, 256]
  den = exp(bias) @ exp(K)     [2048, 256]
  Yt  = sigmoid(Q) * num / den
  out = Yt @ f2_w + f2_b       [2048, 256]

The kernel is HBM-read-bound on the load side (~35 MB of f32 inputs at
the ~358 GB/s per-NC limit is a ~100 us floor) and PE-row-bound on the
compute side. Phases run SEQUENTIALLY (A: all K/V groups, B: all Q
chunks, C: all num/den chunks) -- interleaving B/C per chunk was tried
and measured ~7 us/chunk SLOWER from cross-phase psum/ACT contention.
The DMA queue order matches consumption, and no dma_start ever waits on
a staging-slot semaphore (a waiting DMA blocks the whole SWDGE queue):
bias staging uses 16 quarter-chunk tiles with 12 slots so the reuse
waits land after their targets are long dead.

Precision split (hard-won): num is a SIGNED accumulation, so independent
per-element quantization errors pass through at full strength (no sqrt-N
averaging) -- every operand on the num path (x_kv, wk/wv, exp(K)*V,
exp(bias)) must stay bf16. den is an all-positive accumulation and Q only
feeds a sigmoid gate; both tolerate fp8e4:
  - den runs fp8 MatmulPerfMode.DoubleRow (2 k-tiles/instruction, halves
    den's PE rows): exp(bias) gets an extra fp8 copy (DVE) and exp(K) an
    extra fp8 ACT write.
  - Q runs fp8 DoubleRow: x_q^T tiles are written fp8 by the existing
    PSUM->SBUF transpose copies (free); wq is loaded bf16 and converted
    once on ACT (fp8 DMA casts run at HALF the bf16 DMA rate, so no bulk
    fp8 loads).
  - Everything else (K/V, num, f2, all transposes) is bf16. The output
    is stored bf16 (saves HBM writes; host casts back to f32).

Other notes:
- Transposes are regular matmuls vs an identity (stationary = data), NOT
  is_transpose: transpose-mode does not count as PE-busy for the HAM
  clock gate and previously left the PE throttled at 1.2 GHz.
- The epilogue 1+exp(-Q) runs on ACT; the rest (mul/recip/mul) on DVE.
- Chunks 0-2 defer their f2 matmuls into the next chunk's phase C (PE is
  strict FIFO; it would otherwise idle behind the DVE epilogue at the
  chunk boundary). Chunk 3 pipelines epilogue/f2/store in s-halves to
  shorten the end-of-kernel tail.
"""

import os
import numpy as np
from contextlib import ExitStack

import concourse.bass as bass
import concourse.tile as tile
from concourse import bacc, mybir
from concourse.bass_utils import run_bass_kernel_spmd
from concourse.masks import make_identity

F32 = mybir.dt.float32
BF16 = mybir.dt.bfloat16
FP8 = mybir.dt.float8e4

S = 2048   # n_q
T = 2048   # n_kv
D = 1024   # d_q == d_kv
H = 256    # hidden
G = 256    # output dim
P = 128    # partitions
SCH = 512  # s-chunk for phases B/C (one PSUM bank of fp32)
NSB = SCH // P       # 4 row-blocks per chunk
NCH = S // SCH       # 4 chunks
NT = T // P          # 16 t row-blocks
NG = NT // 2         # 8 groups of 2 t-blocks
ND = D // P          # 8 d tiles
NDD = ND // 2        # 4 d-tile pairs (DoubleRow)
NHB = H // P         # 2 h blocks
TQ = T // 4          # bias quarter length along t

DR = mybir.MatmulPerfMode.DoubleRow
AFT = mybir.ActivationFunctionType


def _build(use_wq_b, use_wk_b, use_wv_b, use_f2_b):
    """Build the per-core Bass graph. Returns the compiled Bacc."""
    nc = bacc.Bacc(
        "TRN2",
        target_bir_lowering=False,
        debug=False,
        enable_asserts=False,
        num_devices=8,
    )

    x_q = nc.declare_dram_parameter("x_q", [S, D], F32, isOutput=False)
    x_kv = nc.declare_dram_parameter("x_kv", [T, D], F32, isOutput=False)
    bias = nc.declare_dram_parameter("bias", [S, T], F32, isOutput=False)
    wq_w = nc.declare_dram_parameter("wq_w", [D, H], F32, isOutput=False)
    wk_w = nc.declare_dram_parameter("wk_w", [D, H], F32, isOutput=False)
    wv_w = nc.declare_dram_parameter("wv_w", [D, H], F32, isOutput=False)
    f2_w = nc.declare_dram_parameter("f2_w", [H, G], F32, isOutput=False)
    wq_b = nc.declare_dram_parameter("wq_b", [1, H], F32, isOutput=False) if use_wq_b else None
    wk_b = nc.declare_dram_parameter("wk_b", [1, H], F32, isOutput=False) if use_wk_b else None
    wv_b = nc.declare_dram_parameter("wv_b", [1, H], F32, isOutput=False) if use_wv_b else None
    f2_b = nc.declare_dram_parameter("f2_b", [1, G], F32, isOutput=False) if use_f2_b else None
    out = nc.declare_dram_parameter("out", [S, G], BF16, isOutput=True)

    # DRAM views with the chunk/block structure exposed.
    xq_v = x_q[:].rearrange("(c sb p) d -> c p sb d", p=P, sb=NSB)     # [NCH,P,NSB,D]
    xkv_v = x_kv[:].rearrange("(g j p) d -> g p j d", p=P, j=2)        # [NG,P,2,D]
    bias_v = bias[:].rearrange("(c sb p) (q t) -> c q p sb t", p=P, sb=NSB, q=4)
    out_v = out[:].rearrange("(c sb p) g -> c p sb g", p=P, sb=NSB)

    with tile.TileContext(nc) as tc, ExitStack() as ctx:
        consts = ctx.enter_context(tc.tile_pool(name="consts", bufs=1))

        ident = consts.tile([P, P], BF16)

        # Staging pools sized so the SWDGE queue never waits on a slot.
        xkv_nat = ctx.enter_context(tc.tile_pool(name="xkv_nat", bufs=2 * NG))
        xq_nat = ctx.enter_context(tc.tile_pool(name="xq_nat", bufs=NCH))
        bias_nat = ctx.enter_context(tc.tile_pool(name="bias_nat", bufs=12))
        xT_sb = ctx.enter_context(tc.tile_pool(name="xT_sb", bufs=8))
        xqT2_sb = ctx.enter_context(tc.tile_pool(name="xqT2_sb", bufs=8))
        expw_sb = ctx.enter_context(tc.tile_pool(name="expw_sb", bufs=6))
        expw8_sb = ctx.enter_context(tc.tile_pool(name="expw8_sb", bufs=3))
        epi = ctx.enter_context(tc.tile_pool(name="epi", bufs=3))
        out_sb_pool = ctx.enter_context(tc.tile_pool(name="out_sb", bufs=2))

        # ---- DMA emission order == consumption order ----
        def load_xkv_j(g, j):
            # Per-(group, j) half tiles: halves the first-transpose latency
            # at kernel start.
            t_ = xkv_nat.tile([P, D], BF16, tag="xkv_nat", name="xkv_nat")
            nc.gpsimd.dma_start(t_[:], xkv_v[g][:, j, :])
            return t_

        def load_xq(c):
            t_ = xq_nat.tile([P, NSB, D], BF16, tag="xq_nat", name="xq_nat")
            nc.gpsimd.dma_start(t_[:], xq_v[c])
            return t_

        xkv_tiles = {}
        for j_ in range(2):
            xkv_tiles[(0, j_)] = load_xkv_j(0, j_)
        # Identity built after the first loads so it does not delay the
        # first DMA on the gpsimd queue.
        make_identity(nc, ident[:])
        # wk|wv fused along the free dim -> K/V matmuls share one N=512 rhs.
        # Loaded right after g0 so the first a_mm starts early.
        wkv_sb = consts.tile([P, ND, 2 * H], BF16, tag="w_wkv", name="wkv_sb")
        nc.gpsimd.dma_start(wkv_sb[:, :, 0:H], wk_w[:].rearrange("(dt p) h -> p dt h", p=P))
        nc.gpsimd.dma_start(wkv_sb[:, :, H : 2 * H], wv_w[:].rearrange("(dt p) h -> p dt h", p=P))
        for g_ in range(1, NG):
            for j_ in range(2):
                xkv_tiles[(g_, j_)] = load_xkv_j(g_, j_)
        wq_bf = consts.tile([P, ND, H], BF16, tag="w_wq", name="wq_bf")
        nc.gpsimd.dma_start(wq_bf[:], wq_w[:].rearrange("(dt p) h -> p dt h", p=P))
        wq_sb = consts.tile([P, ND, H], FP8, tag="w_wq8", name="wq_sb")
        f2_sb = consts.tile([P, NHB, G], BF16, tag="w_f2")
        nc.gpsimd.dma_start(f2_sb[:], f2_w[:].rearrange("(ht p) g -> p ht g", p=P))
        # Load order: xq c0-c2, bias c0, xq c3, bias c1-c3. Phase C(0) can
        # then start right as B(0-2) finish; B(3) is emitted between C(0)
        # and C(1) and fills C(0)'s epilogue latency.
        bias_tiles = {}

        def load_bias(c):
            for q_ in range(4):
                t_ = bias_nat.tile([P, NSB, TQ], BF16, tag="bias_nat", name="bias_nat")
                nc.gpsimd.dma_start(t_[:], bias_v[c][q_])
                bias_tiles[(c, q_)] = t_

        xq_tiles = {c_: load_xq(c_) for c_ in range(NCH - 1)}
        load_bias(0)
        xq_tiles[NCH - 1] = load_xq(NCH - 1)
        for c_ in range(1, NCH):
            load_bias(c_)

        bias_vecs = {}
        ones_row = None
        if any(b is not None for b in (wq_b, wk_b, wv_b, f2_b)):
            ones_row = consts.tile([1, SCH], BF16)
            nc.gpsimd.memset(ones_row[:], 1.0)
            for name, b in (("wq", wq_b), ("f2", f2_b)):
                if b is not None:
                    bt = consts.tile([1, H], BF16, tag=f"b_{name}", name="bt")
                    nc.gpsimd.dma_start(bt[:], b[:])
                    bias_vecs[name] = bt
            if wk_b is not None or wv_b is not None:
                bkv = consts.tile([1, 2 * H], BF16, tag="b_kv", name="bkv")
                nc.gpsimd.memset(bkv[:], 0.0)
                if wk_b is not None:
                    nc.gpsimd.dma_start(bkv[:, 0:H], wk_b[:])
                if wv_b is not None:
                    nc.gpsimd.dma_start(bkv[:, H : 2 * H], wv_b[:])
                bias_vecs["kv"] = bkv

        # Long-lived activations.
        enq_pool = ctx.enter_context(tc.tile_pool(name="enq", bufs=NHB))
        enq = [enq_pool.tile([P, S], BF16, tag="enq", name="enq") for _ in range(NHB)]
        ek_pool = ctx.enter_context(tc.tile_pool(name="expk", bufs=NG))
        ek8_pool = ctx.enter_context(tc.tile_pool(name="expk8", bufs=NG))
        ekv_pool = ctx.enter_context(tc.tile_pool(name="ekv", bufs=NG))
        # expk/ekv: [P, 2H] bf16, j-major (lhsT slices for the bf16 num mm).
        # expk8: [P, 2, H] fp8, k-tile-pair layout for the den DoubleRow mm.
        expk = [ek_pool.tile([P, 2 * H], BF16, tag="expk", name="expk") for _ in range(NG)]
        expk8 = [ek8_pool.tile([P, 2, H], FP8, tag="expk8", name="expk8") for _ in range(NG)]
        ekv = [ekv_pool.tile([P, 2 * H], BF16, tag="ekv", name="ekv") for _ in range(NG)]

        # ---------------- Phases A, B, C ----------------
        # Phases A and B(0-2) run under roomy tr/acc pools (4+4 banks);
        # those close and phase C opens trc 2 + nd 4 + f2 2. B(3) is emitted
        # inside the C region and borrows trc/nd.
        ab_ctx = ExitStack()
        psum_tr = ab_ctx.enter_context(tc.tile_pool(name="psum_tr", bufs=4, space="PSUM"))
        psum_acc = ab_ctx.enter_context(tc.tile_pool(name="psum_acc", bufs=4, space="PSUM"))
        if True:
            # Phase A: K/V, exp(K), exp(K)*V, two t-blocks per group.
            def phase_a(g):
                ps_kv = [psum_acc.tile([P, 2 * H], F32, tag="acc", name="ps_kv") for _ in range(2)]
                xkvT = {}

                def a_tr(d, xkvT=xkvT):
                    ps = psum_tr.tile([P, SCH], F32, tag="tr", name="ps_tr")
                    for j in range(2):
                        nc.tensor.matmul(
                            ps[:, j * P : (j + 1) * P],
                            xkv_tiles[(g, j)][:, d * P : (d + 1) * P],
                            ident[:],
                        )
                    t_ = xT_sb.tile([P, 2 * P], BF16, tag="xkvT", name="xkvT")
                    nc.vector.tensor_copy(t_[:], ps[:, 0 : 2 * P])
                    xkvT[d] = t_

                def a_mm(d, ps_kv=ps_kv, xkvT=xkvT):
                    for j in range(2):
                        nc.tensor.matmul(
                            ps_kv[j][:],
                            xkvT[d][:, j * P : (j + 1) * P],
                            wkv_sb[:, d, :],
                            start=(d == 0),
                            stop=(d == ND - 1 and "kv" not in bias_vecs),
                        )

                for d in range(ND):
                    a_tr(d)
                    if d >= 3:
                        a_mm(d - 3)
                a_mm(ND - 3)
                a_mm(ND - 2)
                a_mm(ND - 1)
                if "kv" in bias_vecs:
                    for j in range(2):
                        nc.tensor.matmul(
                            ps_kv[j][:],
                            ones_row[:, 0:P],
                            bias_vecs["kv"][:],
                            start=False,
                            stop=True,
                        )
                for j in range(2):
                    nc.scalar.activation(
                        expk[g][:, j * H : (j + 1) * H], ps_kv[j][:, 0:H], AFT.Exp
                    )
                    nc.scalar.activation(expk8[g][:, j, :], ps_kv[j][:, 0:H], AFT.Exp)
                    nc.vector.tensor_mul(
                        ekv[g][:, j * H : (j + 1) * H],
                        expk[g][:, j * H : (j + 1) * H],
                        ps_kv[j][:, H : 2 * H],
                    )

            # Phase B: Q^T (fp8 DoubleRow) + exp(-Q), one chunk at a time.
            def phase_b(c, tr_pool=None, q_pool=None, q_tag="acc", tr_tag="tr"):
                tr_pool = tr_pool or psum_tr
                q_pool = q_pool or psum_acc
                nat = xq_tiles[c]
                ps_q = [q_pool.tile([P, SCH], F32, tag=q_tag, name="ps_q") for _ in range(NHB)]
                xqT2 = {}

                def b_tr(d, nat=nat, xqT2=xqT2):
                    ps = tr_pool.tile([P, SCH], F32, tag=tr_tag, name="ps_trq")
                    for sb in range(NSB):
                        nc.tensor.matmul(
                            ps[:, sb * P : (sb + 1) * P],
                            nat[:, sb, d * P : (d + 1) * P],
                            ident[:],
                        )
                    if d % 2 == 0:
                        xqT2[d // 2] = xqT2_sb.tile(
                            [P, 2, SCH], FP8, tag="xqT2", name="xqT2"
                        )
                    nc.vector.tensor_copy(xqT2[d // 2][:, d % 2, :], ps[:])

                def b_mm(dd, ps_q=ps_q, xqT2=xqT2):
                    for hb in range(NHB):
                        nc.tensor.matmul(
                            ps_q[hb][:],
                            wq_sb[:, 2 * dd : 2 * dd + 2, hb * P : (hb + 1) * P],
                            xqT2[dd][:],
                            start=(dd == 0),
                            stop=(dd == NDD - 1 and "wq" not in bias_vecs),
                            perf_mode=DR,
                        )

                for d in range(ND):
                    b_tr(d)
                    if d == 3:
                        b_mm(0)
                    elif d == 5:
                        b_mm(1)
                    elif d == 7:
                        b_mm(2)
                b_mm(NDD - 1)
                for hb in range(NHB):
                    if "wq" in bias_vecs:
                        nc.tensor.matmul(
                            ps_q[hb][:],
                            bias_vecs["wq"][:, hb * P : (hb + 1) * P],
                            ones_row[:],
                            start=False,
                            stop=True,
                        )
                    nc.scalar.activation(
                        enq[hb][:, c * SCH : (c + 1) * SCH],
                        ps_q[hb][:],
                        AFT.Exp,
                        scale=-1.0,
                    )

            def emit_f2(ytT, c):
                # Deferred f2 projection + store for chunks 0-2.
                out_sb = out_sb_pool.tile([P, NSB, G], BF16, tag="out_sb", name="out_sb")
                for sb in range(NSB):
                    ps_f = psum_f2.tile([P, G], F32, tag="f2", name="ps_f")
                    for hb in range(NHB):
                        nc.tensor.matmul(
                            ps_f[:],
                            ytT[hb][:, sb * P : (sb + 1) * P],
                            f2_sb[:, hb, :],
                            start=(hb == 0),
                            stop=(hb == NHB - 1 and "f2" not in bias_vecs),
                        )
                    if "f2" in bias_vecs:
                        nc.tensor.matmul(
                            ps_f[:],
                            ones_row[:, 0:P],
                            bias_vecs["f2"][:],
                            start=False,
                            stop=True,
                        )
                    nc.vector.tensor_copy(out_sb[:, sb, :], ps_f[:])
                nc.sync.dma_start(out_v[c], out_sb[:])

            pending_f2 = [None]

            # Phase C: exp(bias)^T, num (bf16) / den (fp8 DR), epilogue.
            def phase_c(c):
                ps_num = [psum_nd.tile([P, SCH], F32, tag="nd", name="ps_num") for _ in range(NHB)]
                ps_den = [psum_nd.tile([P, SCH], F32, tag="nd", name="ps_den") for _ in range(NHB)]
                expw = []
                expw8 = {}

                def c_tr(tb, expw=expw, expw8=expw8):
                    nath = bias_tiles[(c, tb // 4)]
                    tloc = (tb % 4) * P
                    ps = psum_trc.tile([P, SCH], F32, tag="trc", name="ps_trc")
                    for sb in range(NSB):
                        nc.tensor.matmul(
                            ps[:, sb * P : (sb + 1) * P],
                            nath[:, sb, tloc : tloc + P],
                            ident[:],
                        )
                    sbuf = expw_sb.tile([P, SCH], BF16, tag="expw", name="expw")
                    nc.scalar.activation(sbuf[:], ps[:], AFT.Exp)
                    expw.append(sbuf)
                    # fp8 copy for the den DoubleRow matmul.
                    g = tb // 2
                    if tb % 2 == 0:
                        expw8[g] = expw8_sb.tile([P, 2, SCH], FP8, tag="expw8", name="expw8")
                    nc.vector.tensor_copy(expw8[g][:, tb % 2, :], sbuf[:])

                def c_num(tb, ps_num=ps_num, expw=expw):
                    g, j = tb // 2, tb % 2
                    rhs = expw[tb][:]
                    for hb in range(NHB):
                        lo = j * H + hb * P
                        nc.tensor.matmul(
                            ps_num[hb][:],
                            ekv[g][:, lo : lo + P],
                            rhs,
                            start=(tb == 0),
                            stop=(tb == NT - 1),
                        )

                def c_den(g, ps_den=ps_den, expw8=expw8):
                    for hb in range(NHB):
                        nc.tensor.matmul(
                            ps_den[hb][:],
                            expk8[g][:, :, hb * P : (hb + 1) * P],
                            expw8[g][:],
                            start=(g == 0),
                            stop=(g == NG - 1),
                            perf_mode=DR,
                        )

                for tb in range(NT):
                    c_tr(tb)
                    if tb == 3 and pending_f2[0] is not None:
                        emit_f2(*pending_f2[0])
                        pending_f2[0] = None
                    if tb >= 2:
                        c_num(tb - 2)
                    if tb >= 3 and tb % 2 == 1:
                        c_den((tb - 3) // 2)
                c_num(NT - 2)
                c_num(NT - 1)
                c_den(NG - 1)

                if c < NCH - 1:
                    # Epilogue: Yt^T = num^T / (den^T * (1 + exp(-Q)^T)).
                    ytT = []
                    for hb in range(NHB):
                        t1 = epi.tile([P, SCH], BF16, tag="t1", name="t1")
                        nc.scalar.add(t1[:], enq[hb][:, c * SCH : (c + 1) * SCH], 1.0)
                        d2 = epi.tile([P, SCH], F32, tag="d2", name="d2", bufs=2)
                        nc.vector.tensor_mul(d2[:], t1[:], ps_den[hb][:])
                        rec = epi.tile([P, SCH], F32, tag="rec", name="rec", bufs=2)
                        nc.vector.reciprocal_approx_fast(rec[:], d2[:])
                        yt = epi.tile([P, SCH], BF16, tag="yt", name="yt", bufs=4)
                        nc.vector.tensor_mul(yt[:], rec[:], ps_num[hb][:])
                        ytT.append(yt)
                    pending_f2[0] = (ytT, c)
                else:
                    # Last chunk: pipeline epilogue/f2/store in s-halves to
                    # shorten the end-of-kernel tail.
                    for h in range(2):
                        sl = slice(h * 2 * P, (h + 1) * 2 * P)
                        yth = []
                        for hb in range(NHB):
                            t1 = epi.tile([P, 2 * P], BF16, tag="t1h", name="t1h")
                            nc.scalar.add(
                                t1[:],
                                enq[hb][:, c * SCH + h * 2 * P : c * SCH + (h + 1) * 2 * P],
                                1.0,
                            )
                            d2 = epi.tile([P, 2 * P], F32, tag="d2h", name="d2h", bufs=2)
                            nc.vector.tensor_mul(d2[:], t1[:], ps_den[hb][:, sl])
                            rec = epi.tile([P, 2 * P], F32, tag="rech", name="rech", bufs=2)
                            nc.vector.reciprocal_approx_fast(rec[:], d2[:])
                            yt = epi.tile([P, 2 * P], BF16, tag="yth", name="yth")
                            nc.vector.tensor_mul(yt[:], rec[:], ps_num[hb][:, sl])
                            yth.append(yt)
                        out_sb = out_sb_pool.tile([P, 2, G], BF16, tag="out_h", name="out_h")
                        for k in range(2):
                            ps_f = psum_f2.tile([P, G], F32, tag="f2", name="ps_fh")
                            for hb in range(NHB):
                                nc.tensor.matmul(
                                    ps_f[:],
                                    yth[hb][:, k * P : (k + 1) * P],
                                    f2_sb[:, hb, :],
                                    start=(hb == 0),
                                    stop=(hb == NHB - 1 and "f2" not in bias_vecs),
                                )
                            if "f2" in bias_vecs:
                                nc.tensor.matmul(
                                    ps_f[:],
                                    ones_row[:, 0:P],
                                    bias_vecs["f2"][:],
                                    start=False,
                                    stop=True,
                                )
                            nc.vector.tensor_copy(out_sb[:, k, :], ps_f[:])
                        nc.sync.dma_start(out_v[c][:, 2 * h : 2 * h + 2, :], out_sb[:])

            for g_ in range(NG):
                phase_a(g_)
            # wq bf16 -> fp8 conversion: emitted after phase A so it does not
            # block the ACT FIFO (its input DMA lands ~40us in).
            nc.scalar.copy(wq_sb[:], wq_bf[:])
            for c_ in range(NCH - 1):
                phase_b(c_)
            ab_ctx.close()
            # C pools: trc 2 + nd 4 + f2 2 = 8 banks. B(3) is emitted after
            # C(0) -- it fills C(0)'s epilogue latency, and its xq arrives
            # after bias c0 in the load stream -- borrowing trc/nd psum.
            with (
                tc.tile_pool(name="psum_trc", bufs=2, space="PSUM") as psum_trc,
                tc.tile_pool(name="psum_nd", bufs=4, space="PSUM") as psum_nd,
                tc.tile_pool(name="psum_f2", bufs=2, space="PSUM") as psum_f2,
            ):
                phase_c(0)
                phase_b(NCH - 1, tr_pool=psum_trc, q_pool=psum_nd, q_tag="nd", tr_tag="trc")
                for c_ in range(1, NCH):
                    phase_c(c_)

    nc.compile()
    return nc


_CACHE = {}


def _get_nc(use_wq_b, use_wk_b, use_wv_b, use_f2_b):
    key = (use_wq_b, use_wk_b, use_wv_b, use_f2_b)
    if key not in _CACHE:
        _CACHE[key] = _build(*key)
    return _CACHE[key]


def kernel(x_q, x_kv, bias, wq_w, wq_b, wk_w, wk_b, wv_w, wv_b, f2_w, f2_b,
           _trace=False, _trace_kwargs=None):
    x_q = np.ascontiguousarray(np.asarray(x_q, dtype=np.float32))
    x_kv = np.ascontiguousarray(np.asarray(x_kv, dtype=np.float32))
    bias = np.ascontiguousarray(np.asarray(bias, dtype=np.float32))
    wq_w = np.ascontiguousarray(np.asarray(wq_w, dtype=np.float32))
    wk_w = np.ascontiguousarray(np.asarray(wk_w, dtype=np.float32))
    wv_w = np.ascontiguousarray(np.asarray(wv_w, dtype=np.float32))
    f2_w = np.ascontiguousarray(np.asarray(f2_w, dtype=np.float32))
    wq_b = np.asarray(wq_b, dtype=np.float32)
    wk_b = np.asarray(wk_b, dtype=np.float32)
    wv_b = np.asarray(wv_b, dtype=np.float32)
    f2_b = np.asarray(f2_b, dtype=np.float32)

    use_b = tuple(bool(np.any(b)) for b in (wq_b, wk_b, wv_b, f2_b))
    nc = _get_nc(*use_b)

    n_cores = 8
    in_maps = []
    for i in range(n_cores):
        m = {
            "x_q": x_q[i],
            "x_kv": x_kv[i],
            "bias": bias[i],
            "wq_w": wq_w,
            "wk_w": wk_w,
            "wv_w": wv_w,
            "f2_w": f2_w,
        }
        if use_b[0]:
            m["wq_b"] = wq_b.reshape(1, H)
        if use_b[1]:
            m["wk_b"] = wk_b.reshape(1, H)
        if use_b[2]:
            m["wv_b"] = wv_b.reshape(1, H)
        if use_b[3]:
            m["f2_b"] = f2_b.reshape(1, G)
        in_maps.append(m)

    if not _trace:
        # The NTFF trace hook is unavailable outside the dev harness; make
        # sure a stray BASS_TRACE env var cannot route us onto that path.
        os.environ["BASS_NEVER_TRACE"] = "1"
    else:
        os.environ.pop("BASS_NEVER_TRACE", None)
    res = run_bass_kernel_spmd(
        nc, in_maps, list(range(n_cores)), trace=_trace, **(_trace_kwargs or {})
    )
    out = np.stack(
        [np.asarray(res.results[i]["out"]).astype(np.float32) for i in range(n_cores)],
        axis=0,
    )
    if _trace:
        return out, res
    return out


# revision 22
# speedup vs baseline: 1.1139x; 1.0263x over previous
"""AFT-Full attention kernel for 8 TRN2 NeuronCores.

Data-parallel over batch B=8 (one batch element per core). Per core:
  Q = x_q @ wq + wq_b          [2048, 256]
  K = x_kv @ wk + wk_b         [2048, 256]
  V = x_kv @ wv + wv_b         [2048, 256]
  num = exp(bias) @ (exp(K)*V) [2048, 256]
  den = exp(bias) @ exp(K)     [2048, 256]
  Yt  = sigmoid(Q) * num / den
  out = Yt @ f2_w + f2_b       [2048, 256]

The kernel is HBM-read-bound on the load side (~35 MB of f32 inputs at
the ~358 GB/s per-NC limit is a ~100 us floor) and PE-row-bound on the
compute side. Phases run SEQUENTIALLY (A: all K/V groups, B: all Q
chunks, C: all num/den chunks) -- interleaving B/C per chunk was tried
and measured ~7 us/chunk SLOWER from cross-phase psum/ACT contention.
The DMA queue order matches consumption, and no dma_start ever waits on
a staging-slot semaphore (a waiting DMA blocks the whole SWDGE queue):
bias staging uses 16 quarter-chunk tiles with 12 slots so the reuse
waits land after their targets are long dead.

Precision split (hard-won): num is a SIGNED accumulation, so independent
per-element quantization errors pass through at full strength (no sqrt-N
averaging) -- every operand on the num path (x_kv, wk/wv, exp(K)*V,
exp(bias)) must stay bf16. den is an all-positive accumulation and Q only
feeds a sigmoid gate; both tolerate fp8e4:
  - den runs fp8 MatmulPerfMode.DoubleRow (2 k-tiles/instruction, halves
    den's PE rows): exp(bias) gets an extra fp8 copy (DVE) and exp(K) an
    extra fp8 ACT write.
  - Q runs fp8 DoubleRow: x_q^T tiles are written fp8 by the existing
    PSUM->SBUF transpose copies (free); wq is loaded bf16 and converted
    once on ACT (fp8 DMA casts run at HALF the bf16 DMA rate, so no bulk
    fp8 loads).
  - Everything else (K/V, num, f2, all transposes) is bf16. The output
    is stored bf16 (saves HBM writes; host casts back to f32).

Other notes:
- Transposes are regular matmuls vs an identity (stationary = data), NOT
  is_transpose: transpose-mode does not count as PE-busy for the HAM
  clock gate and previously left the PE throttled at 1.2 GHz.
- The epilogue 1+exp(-Q) runs on ACT; the rest (mul/recip/mul) on DVE.
- Chunks 0-2 defer their f2 matmuls into the next chunk's phase C (PE is
  strict FIFO; it would otherwise idle behind the DVE epilogue at the
  chunk boundary). Chunk 3 pipelines epilogue/f2/store in s-halves to
  shorten the end-of-kernel tail.
"""

import os
import numpy as np
from contextlib import ExitStack

import concourse.bass as bass
import concourse.tile as tile
from concourse import bacc, mybir
from concourse.bass_utils import run_bass_kernel_spmd
from concourse.masks import make_identity

F32 = mybir.dt.float32
BF16 = mybir.dt.bfloat16
FP8 = mybir.dt.float8e4

S = 2048   # n_q
T = 2048   # n_kv
D = 1024   # d_q == d_kv
H = 256    # hidden
G = 256    # output dim
P = 128    # partitions
SCH = 512  # s-chunk for phases B/C (one PSUM bank of fp32)
NSB = SCH // P       # 4 row-blocks per chunk
NCH = S // SCH       # 4 chunks
NT = T // P          # 16 t row-blocks
NG = NT // 2         # 8 groups of 2 t-blocks
ND = D // P          # 8 d tiles
NDD = ND // 2        # 4 d-tile pairs (DoubleRow)
NHB = H // P         # 2 h blocks
TQ = T // 4          # bias quarter length along t

DR = mybir.MatmulPerfMode.DoubleRow
AFT = mybir.ActivationFunctionType


def _build(use_wq_b, use_wk_b, use_wv_b, use_f2_b):
    """Build the per-core Bass graph. Returns the compiled Bacc."""
    nc = bacc.Bacc(
        "TRN2",
        target_bir_lowering=False,
        debug=False,
        enable_asserts=False,
        num_devices=8,
    )

    x_q = nc.declare_dram_parameter("x_q", [S, D], F32, isOutput=False)
    x_kv = nc.declare_dram_parameter("x_kv", [T, D], F32, isOutput=False)
    bias = nc.declare_dram_parameter("bias", [S, T], F32, isOutput=False)
    wq_w = nc.declare_dram_parameter("wq_w", [D, H], F32, isOutput=False)
    wk_w = nc.declare_dram_parameter("wk_w", [D, H], F32, isOutput=False)
    wv_w = nc.declare_dram_parameter("wv_w", [D, H], F32, isOutput=False)
    f2_w = nc.declare_dram_parameter("f2_w", [H, G], F32, isOutput=False)
    wq_b = nc.declare_dram_parameter("wq_b", [1, H], F32, isOutput=False) if use_wq_b else None
    wk_b = nc.declare_dram_parameter("wk_b", [1, H], F32, isOutput=False) if use_wk_b else None
    wv_b = nc.declare_dram_parameter("wv_b", [1, H], F32, isOutput=False) if use_wv_b else None
    f2_b = nc.declare_dram_parameter("f2_b", [1, G], F32, isOutput=False) if use_f2_b else None
    out = nc.declare_dram_parameter("out", [S, G], BF16, isOutput=True)

    # DRAM views with the chunk/block structure exposed.
    xq_v = x_q[:].rearrange("(c sb p) d -> c p sb d", p=P, sb=NSB)     # [NCH,P,NSB,D]
    xkv_v = x_kv[:].rearrange("(g j p) d -> g p j d", p=P, j=2)        # [NG,P,2,D]
    bias_v = bias[:].rearrange("(c sb p) (q t) -> c q p sb t", p=P, sb=NSB, q=4)
    out_v = out[:].rearrange("(c sb p) g -> c p sb g", p=P, sb=NSB)

    with tile.TileContext(nc) as tc, ExitStack() as ctx:
        consts = ctx.enter_context(tc.tile_pool(name="consts", bufs=1))

        ident = consts.tile([P, P], BF16)

        # Staging pools sized so the SWDGE queue never waits on a slot.
        xkv_nat = ctx.enter_context(tc.tile_pool(name="xkv_nat", bufs=2 * NG))
        xq_nat = ctx.enter_context(tc.tile_pool(name="xq_nat", bufs=NCH))
        bias_nat = ctx.enter_context(tc.tile_pool(name="bias_nat", bufs=12))
        xT_sb = ctx.enter_context(tc.tile_pool(name="xT_sb", bufs=8))
        xqT2_sb = ctx.enter_context(tc.tile_pool(name="xqT2_sb", bufs=8))
        expw_sb = ctx.enter_context(tc.tile_pool(name="expw_sb", bufs=6))
        expw8_sb = ctx.enter_context(tc.tile_pool(name="expw8_sb", bufs=3))
        epi = ctx.enter_context(tc.tile_pool(name="epi", bufs=3))
        out_sb_pool = ctx.enter_context(tc.tile_pool(name="out_sb", bufs=2))

        # ---- DMA emission order == consumption order ----
        def load_xkv_j(g, j):
            # Per-(group, j) half tiles: halves the first-transpose latency
            # at kernel start.
            t_ = xkv_nat.tile([P, D], BF16, tag="xkv_nat", name="xkv_nat")
            nc.gpsimd.dma_start(t_[:], xkv_v[g][:, j, :])
            return t_

        def load_xq(c):
            t_ = xq_nat.tile([P, NSB, D], BF16, tag="xq_nat", name="xq_nat")
            nc.gpsimd.dma_start(t_[:], xq_v[c])
            return t_

        xkv_tiles = {}
        for j_ in range(2):
            xkv_tiles[(0, j_)] = load_xkv_j(0, j_)
        # Identity built after the first loads so it does not delay the
        # first DMA on the gpsimd queue.
        make_identity(nc, ident[:])
        # wk|wv fused along the free dim -> K/V matmuls share one N=512 rhs.
        # Loaded right after g0 so the first a_mm starts early.
        wkv_sb = consts.tile([P, ND, 2 * H], BF16, tag="w_wkv", name="wkv_sb")
        nc.gpsimd.dma_start(wkv_sb[:, :, 0:H], wk_w[:].rearrange("(dt p) h -> p dt h", p=P))
        nc.gpsimd.dma_start(wkv_sb[:, :, H : 2 * H], wv_w[:].rearrange("(dt p) h -> p dt h", p=P))
        for g_ in range(1, NG):
            for j_ in range(2):
                xkv_tiles[(g_, j_)] = load_xkv_j(g_, j_)
        wq_bf = consts.tile([P, ND, H], BF16, tag="w_wq", name="wq_bf")
        nc.gpsimd.dma_start(wq_bf[:], wq_w[:].rearrange("(dt p) h -> p dt h", p=P))
        wq_sb = consts.tile([P, ND, H], FP8, tag="w_wq8", name="wq_sb")
        f2_sb = consts.tile([P, NHB, G], BF16, tag="w_f2")
        nc.gpsimd.dma_start(f2_sb[:], f2_w[:].rearrange("(ht p) g -> p ht g", p=P))
        # Load order: xq c0-c2, bias c0, xq c3, bias c1-c3. Phase C(0) can
        # then start right as B(0-2) finish; B(3) is emitted between C(0)
        # and C(1) and fills C(0)'s epilogue latency.
        bias_tiles = {}

        def load_bias(c):
            for q_ in range(4):
                t_ = bias_nat.tile([P, NSB, TQ], BF16, tag="bias_nat", name="bias_nat")
                nc.gpsimd.dma_start(t_[:], bias_v[c][q_])
                bias_tiles[(c, q_)] = t_

        xq_tiles = {c_: load_xq(c_) for c_ in range(NCH)}
        for c_ in range(NCH):
            load_bias(c_)

        bias_vecs = {}
        ones_row = None
        if any(b is not None for b in (wq_b, wk_b, wv_b, f2_b)):
            ones_row = consts.tile([1, SCH], BF16)
            nc.gpsimd.memset(ones_row[:], 1.0)
            for name, b in (("wq", wq_b), ("f2", f2_b)):
                if b is not None:
                    bt = consts.tile([1, H], BF16, tag=f"b_{name}", name="bt")
                    nc.gpsimd.dma_start(bt[:], b[:])
                    bias_vecs[name] = bt
            if wk_b is not None or wv_b is not None:
                bkv = consts.tile([1, 2 * H], BF16, tag="b_kv", name="bkv")
                nc.gpsimd.memset(bkv[:], 0.0)
                if wk_b is not None:
                    nc.gpsimd.dma_start(bkv[:, 0:H], wk_b[:])
                if wv_b is not None:
                    nc.gpsimd.dma_start(bkv[:, H : 2 * H], wv_b[:])
                bias_vecs["kv"] = bkv

        # Long-lived activations.
        enq_pool = ctx.enter_context(tc.tile_pool(name="enq", bufs=NHB))
        enq = [enq_pool.tile([P, S], BF16, tag="enq", name="enq") for _ in range(NHB)]
        ek_pool = ctx.enter_context(tc.tile_pool(name="expk", bufs=NG))
        ek8_pool = ctx.enter_context(tc.tile_pool(name="expk8", bufs=NG))
        ekv_pool = ctx.enter_context(tc.tile_pool(name="ekv", bufs=NG))
        # expk/ekv: [P, 2H] bf16, j-major (lhsT slices for the bf16 num mm).
        # expk8: [P, 2, H] fp8, k-tile-pair layout for the den DoubleRow mm.
        expk = [ek_pool.tile([P, 2 * H], BF16, tag="expk", name="expk") for _ in range(NG)]
        expk8 = [ek8_pool.tile([P, 2, H], FP8, tag="expk8", name="expk8") for _ in range(NG)]
        ekv = [ekv_pool.tile([P, 2 * H], BF16, tag="ekv", name="ekv") for _ in range(NG)]

        # ---------------- Phases A, B, C ----------------
        # Phases A and B(0-2) run under roomy tr/acc pools (4+4 banks);
        # those close and phase C opens trc 2 + nd 4 + f2 2. B(3) is emitted
        # inside the C region and borrows trc/nd.
        ab_ctx = ExitStack()
        psum_tr = ab_ctx.enter_context(tc.tile_pool(name="psum_tr", bufs=4, space="PSUM"))
        psum_acc = ab_ctx.enter_context(tc.tile_pool(name="psum_acc", bufs=4, space="PSUM"))
        if True:
            # Phase A: K/V, exp(K), exp(K)*V, two t-blocks per group.
            def phase_a(g):
                ps_kv = [psum_acc.tile([P, 2 * H], F32, tag="acc", name="ps_kv") for _ in range(2)]
                xkvT = {}

                def a_tr(d, xkvT=xkvT):
                    ps = psum_tr.tile([P, SCH], F32, tag="tr", name="ps_tr")
                    for j in range(2):
                        nc.tensor.matmul(
                            ps[:, j * P : (j + 1) * P],
                            xkv_tiles[(g, j)][:, d * P : (d + 1) * P],
                            ident[:],
                        )
                    t_ = xT_sb.tile([P, 2 * P], BF16, tag="xkvT", name="xkvT")
                    nc.vector.tensor_copy(t_[:], ps[:, 0 : 2 * P])
                    xkvT[d] = t_

                def a_mm(d, ps_kv=ps_kv, xkvT=xkvT):
                    for j in range(2):
                        nc.tensor.matmul(
                            ps_kv[j][:],
                            xkvT[d][:, j * P : (j + 1) * P],
                            wkv_sb[:, d, :],
                            start=(d == 0),
                            stop=(d == ND - 1 and "kv" not in bias_vecs),
                        )

                for d in range(ND):
                    a_tr(d)
                    if d >= 3:
                        a_mm(d - 3)
                a_mm(ND - 3)
                a_mm(ND - 2)
                a_mm(ND - 1)
                if "kv" in bias_vecs:
                    for j in range(2):
                        nc.tensor.matmul(
                            ps_kv[j][:],
                            ones_row[:, 0:P],
                            bias_vecs["kv"][:],
                            start=False,
                            stop=True,
                        )
                for j in range(2):
                    nc.scalar.activation(
                        expk[g][:, j * H : (j + 1) * H], ps_kv[j][:, 0:H], AFT.Exp
                    )
                    nc.scalar.activation(expk8[g][:, j, :], ps_kv[j][:, 0:H], AFT.Exp)
                    nc.vector.tensor_mul(
                        ekv[g][:, j * H : (j + 1) * H],
                        expk[g][:, j * H : (j + 1) * H],
                        ps_kv[j][:, H : 2 * H],
                    )

            # Phase B: Q^T (fp8 DoubleRow) + exp(-Q), one chunk at a time.
            def phase_b(c, tr_pool=None, q_pool=None, q_tag="acc", tr_tag="tr"):
                tr_pool = tr_pool or psum_tr
                q_pool = q_pool or psum_acc
                nat = xq_tiles[c]
                ps_q = [q_pool.tile([P, SCH], F32, tag=q_tag, name="ps_q") for _ in range(NHB)]
                xqT2 = {}

                def b_tr(d, nat=nat, xqT2=xqT2):
                    ps = tr_pool.tile([P, SCH], F32, tag=tr_tag, name="ps_trq")
                    for sb in range(NSB):
                        nc.tensor.matmul(
                            ps[:, sb * P : (sb + 1) * P],
                            nat[:, sb, d * P : (d + 1) * P],
                            ident[:],
                        )
                    if d % 2 == 0:
                        xqT2[d // 2] = xqT2_sb.tile(
                            [P, 2, SCH], FP8, tag="xqT2", name="xqT2"
                        )
                    nc.vector.tensor_copy(xqT2[d // 2][:, d % 2, :], ps[:])

                def b_mm(dd, ps_q=ps_q, xqT2=xqT2):
                    for hb in range(NHB):
                        nc.tensor.matmul(
                            ps_q[hb][:],
                            wq_sb[:, 2 * dd : 2 * dd + 2, hb * P : (hb + 1) * P],
                            xqT2[dd][:],
                            start=(dd == 0),
                            stop=(dd == NDD - 1 and "wq" not in bias_vecs),
                            perf_mode=DR,
                        )

                for d in range(ND):
                    b_tr(d)
                    if d == 3:
                        b_mm(0)
                    elif d == 5:
                        b_mm(1)
                    elif d == 7:
                        b_mm(2)
                b_mm(NDD - 1)
                for hb in range(NHB):
                    if "wq" in bias_vecs:
                        nc.tensor.matmul(
                            ps_q[hb][:],
                            bias_vecs["wq"][:, hb * P : (hb + 1) * P],
                            ones_row[:],
                            start=False,
                            stop=True,
                        )
                    nc.scalar.activation(
                        enq[hb][:, c * SCH : (c + 1) * SCH],
                        ps_q[hb][:],
                        AFT.Exp,
                        scale=-1.0,
                    )

            def emit_f2(ytT, c):
                # Deferred f2 projection + store for chunks 0-2.
                out_sb = out_sb_pool.tile([P, NSB, G], BF16, tag="out_sb", name="out_sb")
                for sb in range(NSB):
                    ps_f = psum_f2.tile([P, G], F32, tag="f2", name="ps_f")
                    for hb in range(NHB):
                        nc.tensor.matmul(
                            ps_f[:],
                            ytT[hb][:, sb * P : (sb + 1) * P],
                            f2_sb[:, hb, :],
                            start=(hb == 0),
                            stop=(hb == NHB - 1 and "f2" not in bias_vecs),
                        )
                    if "f2" in bias_vecs:
                        nc.tensor.matmul(
                            ps_f[:],
                            ones_row[:, 0:P],
                            bias_vecs["f2"][:],
                            start=False,
                            stop=True,
                        )
                    nc.vector.tensor_copy(out_sb[:, sb, :], ps_f[:])
                nc.sync.dma_start(out_v[c], out_sb[:])

            pending_f2 = [None]

            # Phase C: exp(bias)^T, num (bf16) / den (fp8 DR), epilogue.
            def phase_c(c):
                ps_num = [psum_nd.tile([P, SCH], F32, tag="nd", name="ps_num") for _ in range(NHB)]
                ps_den = [psum_nd.tile([P, SCH], F32, tag="nd", name="ps_den") for _ in range(NHB)]
                expw = []
                expw8 = {}

                def c_tr(tb, expw=expw, expw8=expw8):
                    nath = bias_tiles[(c, tb // 4)]
                    tloc = (tb % 4) * P
                    ps = psum_trc.tile([P, SCH], F32, tag="trc", name="ps_trc")
                    for sb in range(NSB):
                        nc.tensor.matmul(
                            ps[:, sb * P : (sb + 1) * P],
                            nath[:, sb, tloc : tloc + P],
                            ident[:],
                        )
                    sbuf = expw_sb.tile([P, SCH], BF16, tag="expw", name="expw")
                    nc.scalar.activation(sbuf[:], ps[:], AFT.Exp)
                    expw.append(sbuf)
                    # fp8 copy for the den DoubleRow matmul.
                    g = tb // 2
                    if tb % 2 == 0:
                        expw8[g] = expw8_sb.tile([P, 2, SCH], FP8, tag="expw8", name="expw8")
                    nc.vector.tensor_copy(expw8[g][:, tb % 2, :], sbuf[:])

                def c_num(tb, ps_num=ps_num, expw=expw):
                    g, j = tb // 2, tb % 2
                    rhs = expw[tb][:]
                    for hb in range(NHB):
                        lo = j * H + hb * P
                        nc.tensor.matmul(
                            ps_num[hb][:],
                            ekv[g][:, lo : lo + P],
                            rhs,
                            start=(tb == 0),
                            stop=(tb == NT - 1),
                        )

                def c_den(g, ps_den=ps_den, expw8=expw8):
                    for hb in range(NHB):
                        nc.tensor.matmul(
                            ps_den[hb][:],
                            expk8[g][:, :, hb * P : (hb + 1) * P],
                            expw8[g][:],
                            start=(g == 0),
                            stop=(g == NG - 1),
                            perf_mode=DR,
                        )

                for tb in range(NT):
                    c_tr(tb)
                    if tb == 3 and pending_f2[0] is not None:
                        emit_f2(*pending_f2[0])
                        pending_f2[0] = None
                    if tb >= 2:
                        c_num(tb - 2)
                    if tb >= 3 and tb % 2 == 1:
                        c_den((tb - 3) // 2)
                c_num(NT - 2)
                c_num(NT - 1)
                c_den(NG - 1)

                if c < NCH - 1:
                    # Epilogue: Yt^T = num^T / (den^T * (1 + exp(-Q)^T)).
                    ytT = []
                    for hb in range(NHB):
                        t1 = epi.tile([P, SCH], BF16, tag="t1", name="t1")
                        nc.scalar.add(t1[:], enq[hb][:, c * SCH : (c + 1) * SCH], 1.0)
                        d2 = epi.tile([P, SCH], F32, tag="d2", name="d2", bufs=2)
                        nc.vector.tensor_mul(d2[:], t1[:], ps_den[hb][:])
                        rec = epi.tile([P, SCH], F32, tag="rec", name="rec", bufs=2)
                        nc.vector.reciprocal_approx_fast(rec[:], d2[:])
                        yt = epi.tile([P, SCH], BF16, tag="yt", name="yt", bufs=4)
                        nc.vector.tensor_mul(yt[:], rec[:], ps_num[hb][:])
                        ytT.append(yt)
                    pending_f2[0] = (ytT, c)
                else:
                    # Last chunk: pipeline epilogue/f2/store in s-halves to
                    # shorten the end-of-kernel tail.
                    for h in range(2):
                        sl = slice(h * 2 * P, (h + 1) * 2 * P)
                        yth = []
                        for hb in range(NHB):
                            t1 = epi.tile([P, 2 * P], BF16, tag="t1h", name="t1h")
                            nc.scalar.add(
                                t1[:],
                                enq[hb][:, c * SCH + h * 2 * P : c * SCH + (h + 1) * 2 * P],
                                1.0,
                            )
                            d2 = epi.tile([P, 2 * P], F32, tag="d2h", name="d2h", bufs=2)
                            nc.vector.tensor_mul(d2[:], t1[:], ps_den[hb][:, sl])
                            rec = epi.tile([P, 2 * P], F32, tag="rech", name="rech", bufs=2)
                            nc.vector.reciprocal_approx_fast(rec[:], d2[:])
                            yt = epi.tile([P, 2 * P], BF16, tag="yth", name="yth")
                            nc.vector.tensor_mul(yt[:], rec[:], ps_num[hb][:, sl])
                            yth.append(yt)
                        out_sb = out_sb_pool.tile([P, 2, G], BF16, tag="out_h", name="out_h")
                        for k in range(2):
                            ps_f = psum_f2.tile([P, G], F32, tag="f2", name="ps_fh")
                            for hb in range(NHB):
                                nc.tensor.matmul(
                                    ps_f[:],
                                    yth[hb][:, k * P : (k + 1) * P],
                                    f2_sb[:, hb, :],
                                    start=(hb == 0),
                                    stop=(hb == NHB - 1 and "f2" not in bias_vecs),
                                )
                            if "f2" in bias_vecs:
                                nc.tensor.matmul(
                                    ps_f[:],
                                    ones_row[:, 0:P],
                                    bias_vecs["f2"][:],
                                    start=False,
                                    stop=True,
                                )
                            nc.vector.tensor_copy(out_sb[:, k, :], ps_f[:])
                        nc.sync.dma_start(out_v[c][:, 2 * h : 2 * h + 2, :], out_sb[:])

            for g_ in range(NG):
                phase_a(g_)
            # wq bf16 -> fp8 conversion: emitted after phase A so it does not
            # block the ACT FIFO (its input DMA lands ~40us in).
            nc.scalar.copy(wq_sb[:], wq_bf[:])
            for c_ in range(NCH):
                phase_b(c_)
            ab_ctx.close()
            # C pools: trc 2 + nd 4 + f2 2 = 8 banks.
            with (
                tc.tile_pool(name="psum_trc", bufs=2, space="PSUM") as psum_trc,
                tc.tile_pool(name="psum_nd", bufs=4, space="PSUM") as psum_nd,
                tc.tile_pool(name="psum_f2", bufs=2, space="PSUM") as psum_f2,
            ):
                for c_ in range(NCH):
                    phase_c(c_)

    nc.compile()
    return nc


_CACHE = {}


def _get_nc(use_wq_b, use_wk_b, use_wv_b, use_f2_b):
    key = (use_wq_b, use_wk_b, use_wv_b, use_f2_b)
    if key not in _CACHE:
        _CACHE[key] = _build(*key)
    return _CACHE[key]


def kernel(x_q, x_kv, bias, wq_w, wq_b, wk_w, wk_b, wv_w, wv_b, f2_w, f2_b,
           _trace=False, _trace_kwargs=None):
    x_q = np.ascontiguousarray(np.asarray(x_q, dtype=np.float32))
    x_kv = np.ascontiguousarray(np.asarray(x_kv, dtype=np.float32))
    bias = np.ascontiguousarray(np.asarray(bias, dtype=np.float32))
    wq_w = np.ascontiguousarray(np.asarray(wq_w, dtype=np.float32))
    wk_w = np.ascontiguousarray(np.asarray(wk_w, dtype=np.float32))
    wv_w = np.ascontiguousarray(np.asarray(wv_w, dtype=np.float32))
    f2_w = np.ascontiguousarray(np.asarray(f2_w, dtype=np.float32))
    wq_b = np.asarray(wq_b, dtype=np.float32)
    wk_b = np.asarray(wk_b, dtype=np.float32)
    wv_b = np.asarray(wv_b, dtype=np.float32)
    f2_b = np.asarray(f2_b, dtype=np.float32)

    use_b = tuple(bool(np.any(b)) for b in (wq_b, wk_b, wv_b, f2_b))
    nc = _get_nc(*use_b)

    n_cores = 8
    in_maps = []
    for i in range(n_cores):
        m = {
            "x_q": x_q[i],
            "x_kv": x_kv[i],
            "bias": bias[i],
            "wq_w": wq_w,
            "wk_w": wk_w,
            "wv_w": wv_w,
            "f2_w": f2_w,
        }
        if use_b[0]:
            m["wq_b"] = wq_b.reshape(1, H)
        if use_b[1]:
            m["wk_b"] = wk_b.reshape(1, H)
        if use_b[2]:
            m["wv_b"] = wv_b.reshape(1, H)
        if use_b[3]:
            m["f2_b"] = f2_b.reshape(1, G)
        in_maps.append(m)

    if not _trace:
        # The NTFF trace hook is unavailable outside the dev harness; make
        # sure a stray BASS_TRACE env var cannot route us onto that path.
        os.environ["BASS_NEVER_TRACE"] = "1"
    else:
        os.environ.pop("BASS_NEVER_TRACE", None)
    res = run_bass_kernel_spmd(
        nc, in_maps, list(range(n_cores)), trace=_trace, **(_trace_kwargs or {})
    )
    out = np.stack(
        [np.asarray(res.results[i]["out"]).astype(np.float32) for i in range(n_cores)],
        axis=0,
    )
    if _trace:
        return out, res
    return out
